# revision 24
# baseline (speedup 1.0000x reference)
"""NeighborhoodShift2d: stack 49 spatially shifted (zero-padded) copies.

Input  x:  [1, 8, 32, 128, 128]  (B, heads, dim, H, W) fp32
Output y:  [1, 8, 49, 32, 128, 128]  y[:, :, k] = shift(x, OFFSETS[k]) with
zero padding, k enumerating the 7x7 NATTEN stencil (dy major, dx minor).

Sharding: pure data-parallel, one head per NeuronCore (8 heads, 8 cores).

Per-core program. The op is pure HBM-write-bound: 102.8 MB of stores vs
2.1 MB of input. Measured DMA rates here: one HWDGE queue sustains
~388 GB/s, two concurrently-fed queues ~413+ GB/s. Design:

- All data lives on 32 partitions chosen with stride 4 ({0,4,...,124}).
  The SBUF DMA port swizzle (port = ((p>>2)&7)<<1 | ((p>>6)&1)) maps this
  set onto ALL 16 SDMA engines (2 partitions each), so every DMA runs at
  the full rate regardless of which queue issued it.
- x is loaded from HBM exactly ONCE (2.1 MB). In flat (h w) space a
  (dy, dx) shift is a single offset of dy*W + dx floats. Each band image
  sits between 388-float zero pads, so a store for (dy, dx) is a fully
  contiguous FP-float read per channel: the dy shift is a read offset,
  edge rows fall into the zero pads, and every store is 32 contiguous
  64 KB descriptors. No edge-fill DMAs exist.
- Only three images are ever materialized: master (dx=0), b1 = master
  shifted +1 (DVE copy, wrap col W-1 zeroed), bm1 = master shifted -1
  (wrap col 0 zeroed). Bands +2/+3 are stored FROM b1 with read offsets
  +1/+2: the extra wrap zeros they need are exactly b1's columns 0 / 0,1
  which are progressively memset (0.4 us) once the preceding band's
  stores complete - the previously-zeroed wrap columns line up with the
  remaining ones by construction. Symmetrically -2/-3 read bm1 at
  offsets -1/-2 with bm1's columns W-1 / W-2,W-1 progressively zeroed.
  Engine ops run lane-aligned on all 128 partitions (the 96 dead
  partitions carry garbage, harmlessly); engine SBUF ports are disjoint
  from the DMA AXI ports, so prep work costs no DMA bandwidth.
- Scheduling: bands are pushed in global order 0,+1,-1,+2,-2,+3,-3 with
  each band's 7 dy-stores ALTERNATING between the two HWDGE rings
  (sync/scalar). A band's wrap-memset gate depends only on the band two
  positions earlier, so a full intervening band (~35 us of queued DMA)
  separates every gate from its dependency: all gates are pre-satisfied,
  both rings stay non-empty the whole run (dual-queue rate), and the
  last band drains on both rings. Small 32-descriptor stores keep the
  engine round-robin fair so completion semaphores fire promptly.
"""

import numpy as np

import concourse.bass as bass
import concourse.mybir as mybir
from concourse.bass_utils import run_bass_kernel_spmd

B, HEADS, C, H, W = 1, 8, 32, 128, 128
WIN = 7
PAD = 3
K = WIN * WIN
FP = H * W            # flat image floats per channel (16384)
PADF = PAD * W + 4    # zero pad between band images (388 >= 384+2)
SLOT = FP + PADF      # slot pitch (16772)
S2 = PADF + 3 * SLOT  # per-partition floats (50704 = 202816 B)
CFP = C * FP          # one k-slice of y, in floats

# Band order: gates depend only on the band two positions earlier.
BANDS = [0, 1, -1, 2, -2, 3, -3]

# SDMA engine 15 (port 15) is intermittently ~22% slower than the other
# 15 engines (known trn2 quirk); with the balanced layout it accumulates
# a descriptor backlog that drains as a serial 20-60 us tail after every
# other engine finishes. Channels 23 and 31 are the only ones on port-15
# partitions (92, 124). Fix: keep DUPLICATES of those two channels on
# spare partition PAIRS (8 apart, so src/dst strides stay regular) that
# map to eight different other ports; every ~6th store reads ch23/ch31
# from a duplicate pair instead: engine 15 gets 84 instead of 100
# descriptors (matching its degraded speed), and the displaced work
# spreads +2 descriptors per donor port. The 128-lane DVE ops already
# process every partition, so the duplicates get the shifted bands and
# wrap zeroing for free. All pieces keep >=2-partition uniform-stride
# APs: degenerate 1-partition APs go down balance_dma_aps' singular-
# split path, which reroutes and re-chunks descriptors (breaks both
# balance and addressing - measured).
DUP_PAIRS = [(65, 73), (81, 89), (1, 9), (17, 25)]  # (ch23, ch31) parts
DUP_STORES = {2, 8, 14, 20, 26, 32, 38, 44}  # global store indices

_nc_cache = None


def _build_nc():
    f32 = mybir.dt.float32
    nc = bass.Bass()
    x = nc.dram_tensor("x", [C, H, W], f32, kind="ExternalInput")
    y = nc.dram_tensor("y", [K, C, H, W], f32, kind="ExternalOutput")

    # Image start offsets: [pad][img0][pad][img1][pad][img2][pad]
    IMG = [PADF + s * SLOT for s in range(3)]

    # Static store assignment: store j of band i goes to ring (i+j) % 2
    # (0 = sync, 1 = scalar). Cumulative per-ring store counts after each
    # band, for semaphore thresholds.
    ring_of = {}
    cum = {0: [], 1: []}
    tot = {0: 0, 1: 0}
    for i, dx in enumerate(BANDS):
        for j in range(WIN):
            r = (i + j) % 2
            ring_of[(dx, j)] = r
            tot[r] += 3 if i * WIN + j in DUP_STORES else 1
        cum[0].append(tot[0])
        cum[1].append(tot[1])
    N_LOADS = 1 + len(DUP_PAIRS)  # s_ld target = 16 * N_LOADS

    # prep-sem threshold each band's stores must wait for
    PREP_GATE = {0: 1, 1: 2, -1: 3, 2: 4, -2: 5, 3: 6, -3: 7}

    with (
        nc.sbuf_tensor("T", [128, S2], f32) as T,
        nc.semaphore("s_ld") as s_ld,
        nc.semaphore("s_prep") as s_prep,
        nc.semaphore("s_stS") as s_stS,
        nc.semaphore("s_stA") as s_stA,
        nc.Block() as block,
    ):
        def store(eng, dx, j, gi, sem):
            """Store k-slice (dy = j-3, dx). Normally one 32x64KB-
            descriptor DMA; dup-split stores read ch23/ch31 from a
            duplicate partition pair to relieve SDMA engine 15."""
            dy = j - PAD
            slot = 0 if dx == 0 else (1 if dx > 0 else 2)
            e = 0 if dx == 0 else (dx - 1 if dx > 0 else dx + 1)
            k = (dy + PAD) * WIN + (dx + PAD)
            off = IMG[slot] + e + dy * W
            if gi not in DUP_STORES:
                src = bass.AP(T, off, [[4 * S2, C], [1, FP]])
                dst = bass.AP(y, k * CFP, [[FP, C], [1, FP]])
                eng.dma_start(out=dst, in_=src).then_inc(sem, 16)
                return
            p23, _ = DUP_PAIRS[(gi // 6) % len(DUP_PAIRS)]
            pieces = [
                (off, [[4 * S2, 23], [1, FP]], 0, [[FP, 23], [1, FP]]),
                (96 * S2 + off, [[4 * S2, 7], [1, FP]],
                 24 * FP, [[FP, 7], [1, FP]]),
                (p23 * S2 + off, [[8 * S2, 2], [1, FP]],
                 23 * FP, [[8 * FP, 2], [1, FP]]),
            ]
            for so, sap, do, dap in pieces:
                eng.dma_start(
                    out=bass.AP(y, k * CFP + do, dap), in_=bass.AP(T, so, sap)
                ).then_inc(sem, 16)

        def dup_load(eng, p23):
            """Load ch23+ch31 into a duplicate partition pair (2 descs)."""
            eng.dma_start(
                out=bass.AP(T, p23 * S2 + IMG[0], [[8 * S2, 2], [1, FP]]),
                in_=bass.AP(x, 23 * FP, [[8 * FP, 2], [1, FP]]),
            ).then_inc(s_ld, 16)

        def ring_program(eng, my_ring, sem):
            for i, dx in enumerate(BANDS):
                eng.wait_ge(s_prep, PREP_GATE[dx])
                if dx == 0:
                    eng.wait_ge(s_ld, 16 * N_LOADS)
                for j in range(WIN):
                    if ring_of[(dx, j)] == my_ring:
                        store(eng, dx, j, i * WIN + j, sem)
            eng.wait_ge(sem, 16 * tot[my_ring])

        def col_zero(vector, slot, col):
            """Zero column `col` of a band image on all rows/partitions."""
            return vector.memset(
                bass.AP(T, IMG[slot] + col, [[S2, 128], [W, H], [1, 1]]), 0.0
            )

        def band_done(vector, band_idx):
            """Wait until all stores of bands[0..band_idx] completed."""
            vector.wait_ge(s_stS, 16 * cum[0][band_idx])
            vector.wait_ge(s_stA, 16 * cum[1][band_idx])

        @block.vector
        def _(vector):
            # Zero the 4 inter-slot pad strips (~1.6 us).
            vector.memset(
                bass.AP(T, 0, [[S2, 128], [SLOT, 4], [1, PADF]]), 0.0
            ).then_inc(s_prep, 1)                                    # -> 1
            vector.wait_ge(s_ld, 16 * N_LOADS)
            # b1 = master shifted +1 (last read lands in master's zero
            # post-pad), wrap col W-1 zeroed.
            vector.tensor_copy(
                out=bass.AP(T, IMG[1], [[S2, 128], [1, FP]]),
                in_=bass.AP(T, IMG[0] + 1, [[S2, 128], [1, FP]]),
            )
            col_zero(vector, 1, W - 1).then_inc(s_prep, 1)           # -> 2
            # bm1 = master shifted -1, wrap col 0 zeroed.
            vector.tensor_copy(
                out=bass.AP(T, IMG[2], [[S2, 128], [1, FP]]),
                in_=bass.AP(T, IMG[0] - 1, [[S2, 128], [1, FP]]),
            )
            col_zero(vector, 2, 0).then_inc(s_prep, 1)               # -> 3
            # Progressive wrap-column zeroing, each gated on the stores
            # still reading that column having completed.
            band_done(vector, 1)                 # bands 0, +1 stored
            col_zero(vector, 1, 0).then_inc(s_prep, 1)               # -> 4
            band_done(vector, 2)                 # band -1 stored
            col_zero(vector, 2, W - 1).then_inc(s_prep, 1)           # -> 5
            band_done(vector, 3)                 # band +2 stored
            col_zero(vector, 1, 1).then_inc(s_prep, 1)               # -> 6
            band_done(vector, 4)                 # band -2 stored
            col_zero(vector, 2, W - 2).then_inc(s_prep, 1)           # -> 7

        @block.sync
        def _(sync):
            nc.sync.dma_start(
                out=bass.AP(T, IMG[0], [[4 * S2, C], [1, FP]]),
                in_=x.rearrange("c h w -> c (h w)")[:, :],
            ).then_inc(s_ld, 16)
            for p23, _ in DUP_PAIRS[:2]:
                dup_load(nc.sync, p23)
            ring_program(sync, 0, s_stS)

        @block.scalar
        def _(scalar):
            for p23, _ in DUP_PAIRS[2:]:
                dup_load(nc.scalar, p23)
            ring_program(scalar, 1, s_stA)

    return nc


def _get_nc():
    global _nc_cache
    if _nc_cache is None:
        _nc_cache = _build_nc()
    return _nc_cache


def kernel(x: np.ndarray) -> np.ndarray:
    assert x.shape == (B, HEADS, C, H, W), x.shape
    nc = _get_nc()
    in_maps = [
        {"x": np.ascontiguousarray(x[0, h], dtype=np.float32)} for h in range(HEADS)
    ]
    res = run_bass_kernel_spmd(nc, in_maps, core_ids=list(range(HEADS)))
    out = np.stack([res.results[h]["y"] for h in range(HEADS)], axis=0)
    return out[None]  # [1, 8, 49, 32, 128, 128]
